# revision 20
# baseline (speedup 1.0000x reference)
"""Trainium2 Bass kernel for nn_MultiHeadAttention_46188078301212.

Module semantics (replicated from the PyTorch module's quirky reshape):
  P_q = q @ Wq.T + bq  (same for k, v), each [B, 2048, 512]
  Head h takes projection rows [256h, 256h+256) viewed as [2048, 64]
  (row-major), runs standard softmax attention, heads are concatenated
  along hidden (out col block 64h..64h+64) and merged with Wm.T + bm.

Sharding: 8 cores = (batch b in {0,1}) x (head-pair j4 in {0..3}).
Core c handles b = c//4 and global heads {2*j4, 2*j4+1}; each core
emits a [2048, 512] partial (its 2 heads' contribution); host sums 4
partials per batch and adds bm.

Key on-core structure (v2 — row-tiled):
  - Working index q' = 256*(e//64) + s is a permutation of the true
    sequence index s' = 8*s + e//64; the final DMA un-permutes.
  - Scores are computed transposed (kappa on partitions, q on free) as
    CONCURRENT K=64 row-tiled matmul pairs: PE rows 0:63 compute the
    kappa'[0,1024) chunk, rows 64:127 the kappa'[1024,2048) chunk
    (disjoint row-groups run in parallel -> 2 cols/cycle).
  - PV is likewise row-tiled: each 128-kappa chunk contracts as two
    concurrent K=64 matmuls into separate PSUM accumulators Oa/Ob
    (summed during extraction).  V carries an all-ones 65th column so
    softmax denominators ride along.
  - O is normalized by 1/den BEFORE the merge (reciprocal_approx_fast +
    gpsimd partition_broadcast + DVE multiply), which lets the two
    heads' merges pack into single K=128 matmuls (lhsT rows 0:64 =
    head t=0, rows 64:128 = t=1) accumulating directly in PSUM; output
    rows DMA straight from PSUM to DRAM.  Phase order (t,H) =
    (0,0),(1,0),(0,1),(1,1) so both heads of an H-half finish together.
  - All matmuls bf16 (psum f32); probs bf16; exp on ScalarE (scale=1/8).
"""

import numpy as np

HIDDEN = 512
DHEAD = 64
B = 2
S = 2048
NCORES = 8

_CACHE = {}

LAST_EXEC_NS = None
LAST_RESULTS = None


def _build_nc():
    if "nc" in _CACHE:
        return _CACHE["nc"]

    import contextlib
    from collections import deque

    import concourse.mybir as mybir
    import concourse.tile as tile
    from concourse import bacc

    f32 = mybir.dt.float32
    bf16 = mybir.dt.bfloat16
    Exp = mybir.ActivationFunctionType.Exp
    mult = mybir.AluOpType.mult
    add = mybir.AluOpType.add

    nc = bacc.Bacc("TRN2", target_bir_lowering=False)

    # ---- DRAM I/O ----
    d_x = {}
    d_w = {}
    for nm in ("q", "k", "v"):
        d_x[nm] = nc.dram_tensor(f"x{nm}T", [512, 512], bf16, kind="ExternalInput")
        wcols = 1024 if nm == "q" else 512
        d_w[nm] = nc.dram_tensor(f"w{nm}T", [512, wcols], bf16, kind="ExternalInput")
    d_wm = nc.dram_tensor("wmP", [128, 512], bf16, kind="ExternalInput")
    d_bq = nc.dram_tensor("bq8", [128, 8], f32, kind="ExternalInput")
    d_bk = nc.dram_tensor("bk4", [128, 4], f32, kind="ExternalInput")
    d_bv = nc.dram_tensor("bv1", [1, 512], bf16, kind="ExternalInput")
    d_on = nc.dram_tensor("ones1", [1, 128], bf16, kind="ExternalInput")
    d_out = nc.dram_tensor("outp", [2048, 512], f32, kind="ExternalOutput")
    # row s' = 8*j + w  (j = q' mod 256, w = q' // 256)
    d_out_r = d_out.rearrange("(j w) e -> j w e", w=8)

    with tile.TileContext(nc) as tc:
        ctx = contextlib.ExitStack()
        with ctx:
            big = ctx.enter_context(tc.tile_pool(name="big", bufs=8))
            sp = ctx.enter_context(tc.tile_pool(name="sp", bufs=2, space="PSUM"))
            op = ctx.enter_context(tc.tile_pool(name="op", bufs=2, space="PSUM"))
            pp = ctx.enter_context(tc.tile_pool(name="pp", bufs=2, space="PSUM"))
            ptp = ctx.enter_context(tc.tile_pool(name="ptp", bufs=6))
            small = ctx.enter_context(tc.tile_pool(name="small", bufs=1))
            otp = ctx.enter_context(tc.tile_pool(name="otp", bufs=2))
            drp = ctx.enter_context(tc.tile_pool(name="drp", bufs=2))
            rcp = ctx.enter_context(tc.tile_pool(name="rcp", bufs=2))
            osp = ctx.enter_context(tc.tile_pool(name="osp", bufs=4))
            msp = ctx.enter_context(tc.tile_pool(name="msp", bufs=2))

            # ---- tiny warm-up constant ----
            ones_tiny = small.tile([1, 1], f32, tag="ones_tiny")
            nc.vector.memset(ones_tiny, 1.0)
            warm = small.tile([1, 1], f32, tag="warm")
            nc.scalar.activation(warm, ones_tiny, Exp, scale=1.0)

            bq8 = small.tile([128, 8], f32, tag="bq8")
            nc.gpsimd.dma_start(out=bq8, in_=d_bq[:, :])

            xin = {}
            win = {}
            for nm, eng in (("q", nc.sync), ("k", nc.scalar), ("v", nc.gpsimd)):
                xt = big.tile([128, 4, 512], bf16, tag="big", name=f"x{nm}")
                eng.dma_start(out=xt, in_=d_x[nm].rearrange("(t p) d -> p t d", p=128))
                wcols = 1024 if nm == "q" else 512
                wt = big.tile([128, 4, wcols], bf16, tag="big", name=f"w{nm}")
                eng.dma_start(out=wt, in_=d_w[nm].rearrange("(t p) d -> p t d", p=128))
                xin[nm] = xt
                win[nm] = wt

            bk4 = small.tile([128, 4], f32, tag="bk4")
            nc.gpsimd.dma_start(out=bk4, in_=d_bk[:, :])
            bv1 = small.tile([1, 512], bf16, tag="bv1")
            nc.gpsimd.dma_start(out=bv1, in_=d_bv[:, :])
            ones1 = small.tile([1, 128], bf16, tag="ones1")
            nc.gpsimd.dma_start(out=ones1, in_=d_on[:, :])
            wm_sb = small.tile([128, 512], bf16, tag="wm", name="wm")
            nc.gpsimd.dma_start(out=wm_sb, in_=d_wm[:, :])

            # ---- persistent per-head tensors ----
            # QQ: rows 0:64 AND 64:128 both hold Q^T (dup for row tiling)
            QQ = small.tile([128, 2, 2048], bf16, tag="QQ", name="QQ")
            # KT2 rows 0:64 = K^T kappa' [0,1024); rows 64:128 = [1024,2048)
            KT2 = small.tile([128, 2, 1024], bf16, tag="KT2", name="KT2")
            # Vt[t][p, c, hv, f]: chunk jj=(2c+hv); f 0:64 = V, f 64 = 1.0
            Vt = [
                small.tile([128, 8, 2, 65], bf16, tag=f"V{t}", name=f"V{t}")
                for t in range(2)
            ]
            for t in range(2):
                nc.vector.memset(Vt[t][:, :, :, 64:65], 1.0)
            # OT[H]: rows 0:64 = normalized O^T of head t=0, 64:128 = t=1
            OT = [
                otp.tile([128, 1024], bf16, tag=f"OT{i}", name=f"OT{i}")
                for i in range(2)
            ]

            # ---- projection emitters ----
            def proj_q(cq):
                xt, wt = xin["q"], win["q"]
                ps = pp.tile([128, 512], f32, tag="pp", name="psq")
                for D in range(4):
                    nc.tensor.matmul(
                        ps,
                        wt[:, D, 128 * cq : 128 * cq + 128],
                        xt[:, D, :],
                        start=(D == 0),
                        stop=(D == 3),
                    )
                psr = ps.rearrange("p (t s) -> p t s", t=2)
                nc.vector.tensor_scalar_add(
                    QQ[0:64, :, 256 * cq : 256 * cq + 256],
                    psr[0:64, :, :],
                    bq8[0:64, cq : cq + 1],
                )
                nc.vector.tensor_scalar_add(
                    QQ[64:128, :, 256 * cq : 256 * cq + 256],
                    psr[64:128, :, :],
                    bq8[64:128, cq : cq + 1],
                )

            def proj_k(cp):
                xt, wt = xin["k"], win["k"]
                ps = pp.tile([128, 512], f32, tag="pp", name="psk")
                for D in range(4):
                    nc.tensor.matmul(
                        ps,
                        wt[:, D, 128 * cp : 128 * cp + 128],
                        xt[:, D, :],
                        start=(D == 0),
                        stop=(D == 3),
                    )
                nc.vector.tensor_scalar_add(
                    KT2[:, :, 256 * cp : 256 * cp + 256],
                    ps.rearrange("p (t s) -> p t s", t=2),
                    bk4[:, cp : cp + 1],
                )

            def proj_v(St):
                xt, wt = xin["v"], win["v"]
                ps = pp.tile([128, 512], f32, tag="pp", name="psv")
                for D in range(4):
                    nc.tensor.matmul(
                        ps,
                        xt[:, D, 128 * St : 128 * St + 128],
                        wt[:, D, :],
                        start=(D == 0),
                        stop=False,
                    )
                nc.tensor.matmul(
                    ps, ones1[0:1, :], bv1[0:1, :], start=False, stop=True
                )
                t, hv = St // 2, St % 2
                nc.vector.tensor_copy(
                    Vt[t][:, :, hv, 0:64],
                    ps.rearrange("p (c d) -> p c d", c=8),
                )

            # Upfront projections double as the HAM warm-up: 24 back-to-back
            # matmuls from DMA-land keep the PE busy through the cold window.
            proj_todo = deque()
            for cq in range(2):
                proj_q(cq)
            proj_k(0)
            proj_k(1)
            proj_v(0)
            proj_v(1)
            proj_todo.append(lambda: proj_q(2))
            proj_todo.append(lambda: proj_k(2))
            proj_todo.append(lambda: proj_q(3))
            proj_todo.append(lambda: proj_k(3))
            proj_todo.append(lambda: proj_v(2))
            proj_todo.append(lambda: proj_v(3))
            for cq in range(4, 8):
                proj_todo.append(lambda cq=cq: proj_q(cq))

            # ---- attention: 4 phases x 16 slots; slot = (n, jp) n-major ----
            phases = [(0, 0), (1, 0), (0, 1), (1, 1)]
            LAG = 2
            pend = deque()
            extract_pending = deque()
            merge_q = deque()
            O_tiles = {}

            def do_merge(H, ii):
                mp = pp.tile([128, 512], f32, tag="pp", name="mp")
                nc.tensor.matmul(mp, OT[H][:, 128 * ii : 128 * ii + 128], wm_sb)
                ms = msp.tile([128, 512], f32, tag="ms", name="ms")
                nc.vector.tensor_copy(ms, mp)
                j0 = 128 * (ii % 2)
                nc.sync.dma_start(
                    out=d_out_r[j0 : j0 + 128, 4 * H + ii // 2, :], in_=ms
                )

            def do_extract(pi, n):
                t, H = phases[pi]
                Oa, Ob = O_tiles[(pi, n)]
                ca = osp.tile([65, 512], f32, tag="ca", name="ca")
                nc.vector.tensor_copy(ca, Oa[0:65, :])
                dsum = drp.tile([1, 512], f32, tag="ds", name="dsum")
                nc.vector.tensor_tensor(dsum, Ob[64:65, :], ca[64:65, :], add)
                rr = drp.tile([1, 512], f32, tag="rr", name="rr")
                nc.vector.reciprocal_approx_fast(rr, dsum)
                rb = rcp.tile([64, 512], f32, tag="rb", name="rb")
                nc.gpsimd.partition_broadcast(rb, rr)
                osum = osp.tile([64, 512], f32, tag="os", name="osum")
                nc.vector.tensor_tensor(osum, Ob[0:64, :], ca[0:64, :], add)
                ots = OT[H][64 * t : 64 * t + 64, 512 * n : 512 * n + 512]
                nc.vector.tensor_tensor(ots, osum, rb, mult)
                if t == 1:
                    for ii in range(4 * n, 4 * n + 4):
                        merge_q.append(lambda ii=ii, H=H: do_merge(H, ii))

            def issue_pv(pi, n, jp, pt):
                t, H = phases[pi]
                if (pi, n) not in O_tiles:
                    O_tiles[(pi, n)] = (
                        op.tile([65, 512], f32, tag="op", name="Oa"),
                        op.tile([65, 512], f32, tag="op", name="Ob"),
                    )
                Oa, Ob = O_tiles[(pi, n)]
                for ci in (0, 1):
                    jj = jp + 8 * ci
                    c, hv = jj // 2, jj % 2
                    st = jp == 0 and ci == 0
                    sp_ = jp == 7 and ci == 1
                    nc.tensor.matmul(
                        Oa[0:65, :],
                        Vt[t][0:64, c, hv, :],
                        pt[0:64, 512 * ci : 512 * ci + 512],
                        start=st,
                        stop=sp_,
                    )
                    nc.tensor.matmul(
                        Ob[0:65, :],
                        Vt[t][64:128, c, hv, :],
                        pt[64:128, 512 * ci : 512 * ci + 512],
                        start=st,
                        stop=sp_,
                    )
                if jp == 7:
                    extract_pending.append((pi, n))

            for pi, (t, H) in enumerate(phases):
                for s in range(16):
                    n, jp = s // 8, s % 8
                    if proj_todo:
                        proj_todo.popleft()()
                    sT = sp.tile([128, 1024], f32, tag="sp", name="sT")
                    nc.tensor.matmul(
                        sT[:, 0:512],
                        KT2[0:64, t, 128 * jp : 128 * jp + 128],
                        QQ[0:64, t, 1024 * H + 512 * n : 1024 * H + 512 * n + 512],
                    )
                    nc.tensor.matmul(
                        sT[:, 512:1024],
                        KT2[64:128, t, 128 * jp : 128 * jp + 128],
                        QQ[64:128, t, 1024 * H + 512 * n : 1024 * H + 512 * n + 512],
                    )
                    pt = ptp.tile([128, 1024], bf16, tag="pt")
                    nc.scalar.activation(pt, sT, Exp, scale=0.125)
                    while extract_pending:
                        do_extract(*extract_pending.popleft())
                    pend.append((pi, n, jp, pt))
                    if pend and pend[0][0] < pi:
                        issue_pv(*pend.popleft())
                    if len(pend) > LAG:
                        issue_pv(*pend.popleft())
                    if merge_q:
                        merge_q.popleft()()
                    if pi == 3 and merge_q:
                        merge_q.popleft()()
            while pend:
                issue_pv(*pend.popleft())
                while extract_pending:
                    do_extract(*extract_pending.popleft())
            while extract_pending:
                do_extract(*extract_pending.popleft())
            while merge_q:
                merge_q.popleft()()

    nc.finalize()
    _CACHE["nc"] = nc
    return nc


def _prep_in_maps(q, k, v, Wq, Wk, Wv, Wm, bq, bk, bv):
    import ml_dtypes

    f = np.float32
    b16 = ml_dtypes.bfloat16
    # wqT: each 64-col chunk duplicated -> [512, 1024] (col 128c+64r+d)
    WqT = Wq.T.astype(b16)
    WqT = np.ascontiguousarray(
        np.repeat(WqT.reshape(512, 8, 1, 64), 2, axis=2).reshape(512, 1024)
    )
    # wkT: columns paired (c, c+4) -> col 128*cp + 64*h + d = orig (cp+4h)*64+d
    WkT = Wk.T.astype(b16)
    WkT = np.ascontiguousarray(
        WkT.reshape(512, 2, 4, 64).transpose(0, 2, 1, 3).reshape(512, 512)
    )
    WvT = np.ascontiguousarray(Wv.T.astype(b16))
    WmT = Wm.T.astype(f)  # [e_in, e_out]

    t_ = np.ascontiguousarray(bq.astype(f).reshape(8, 64).T)  # [64, 8]
    bq8 = np.ascontiguousarray(np.vstack([t_, t_]))  # [128, 8]
    kk = bk.astype(f).reshape(8, 64)
    bk4 = np.ascontiguousarray(np.concatenate([kk[0:4].T, kk[4:8].T], axis=0))
    bv1 = np.ascontiguousarray(bv.astype(b16).reshape(1, 512))

    in_maps = []
    for c in range(NCORES):
        b_, j4 = c // 4, c % 4
        r0 = 512 * j4
        m = {
            "xqT": np.ascontiguousarray(q[b_, r0 : r0 + 512, :].T.astype(b16)),
            "xkT": np.ascontiguousarray(k[b_, r0 : r0 + 512, :].T.astype(b16)),
            "xvT": np.ascontiguousarray(v[b_, r0 : r0 + 512, :].T.astype(b16)),
            "wqT": WqT,
            "wkT": WkT,
            "wvT": WvT,
            "wmP": np.ascontiguousarray(
                WmT[128 * j4 : 128 * j4 + 128, :].astype(b16)
            ),
            "bq8": bq8,
            "bk4": bk4,
            "bv1": bv1,
            "ones1": np.ones((1, 128), b16),
        }
        in_maps.append(m)
    return in_maps


def _reference_fallback(q, k, v, mask, Wq, Wk, Wv, Wm, bq, bk, bv, bm):
    # Only used if mask is nonzero (spec fills it with zeros).
    f = np.float32
    qh = (q.astype(f) @ Wq.T.astype(f) + bq).reshape(B, 8, S, DHEAD)
    kh = (k.astype(f) @ Wk.T.astype(f) + bk).reshape(B, 8, S, DHEAD)
    vh = (v.astype(f) @ Wv.T.astype(f) + bv).reshape(B, 8, S, DHEAD)
    s = np.einsum("bhqd,bhkd->bhqk", qh, kh) / np.sqrt(np.float32(DHEAD))
    s = np.where(mask, np.float32(-1e9), s)
    s = s - s.max(-1, keepdims=True)
    e = np.exp(s)
    p = e / e.sum(-1, keepdims=True)
    attn = np.einsum("bhqk,bhkd->bhqd", p, vh)
    attn = attn.transpose(0, 2, 1, 3).reshape(B, S, HIDDEN)
    return attn @ Wm.T.astype(f) + bm


def kernel(q, k, v, mask, Wq, Wk, Wv, Wm, bq, bk, bv, bm):
    global LAST_EXEC_NS, LAST_RESULTS
    q, k, v = (np.asarray(a, np.float32) for a in (q, k, v))
    mask = np.asarray(mask)
    Wq, Wk, Wv, Wm = (np.asarray(a, np.float32) for a in (Wq, Wk, Wv, Wm))
    bq, bk, bv, bm = (np.asarray(a, np.float32) for a in (bq, bk, bv, bm))

    if mask.any():
        return _reference_fallback(q, k, v, mask, Wq, Wk, Wv, Wm, bq, bk, bv, bm)

    from concourse.bass_utils import run_bass_kernel_spmd

    nc = _build_nc()
    in_maps = _prep_in_maps(q, k, v, Wq, Wk, Wv, Wm, bq, bk, bv)
    res = run_bass_kernel_spmd(nc, in_maps, list(range(NCORES)))
    LAST_RESULTS = res
    LAST_EXEC_NS = getattr(res, "exec_time_ns", None)

    out = np.zeros((B, S, HIDDEN), np.float32)
    for c in range(NCORES):
        out[c // 4] += res.results[c]["outp"]
    out += bm
    return out


# revision 30
# speedup vs baseline: 1.0080x; 1.0080x over previous
"""Trainium2 Bass kernel for nn_MultiHeadAttention_46188078301212.

Module semantics (replicated from the PyTorch module's quirky reshape):
  P_q = q @ Wq.T + bq  (same for k, v), each [B, 2048, 512]
  Head h takes projection rows [256h, 256h+256) viewed as [2048, 64]
  (row-major), runs standard softmax attention, heads are concatenated
  along hidden (out col block 64h..64h+64) and merged with Wm.T + bm.

Sharding: 8 cores = (batch b in {0,1}) x (head-pair j4 in {0..3}).
Core c handles b = c//4 and global heads {2*j4, 2*j4+1}; each core
emits a [2048, 512] partial (its 2 heads' contribution); host sums 4
partials per batch and adds bm.

Key on-core structure (v2 — row-tiled):
  - Working index q' = 256*(e//64) + s is a permutation of the true
    sequence index s' = 8*s + e//64; the final DMA un-permutes.
  - Scores are computed transposed (kappa on partitions, q on free) as
    CONCURRENT K=64 row-tiled matmul pairs: PE rows 0:63 compute the
    kappa'[0,1024) chunk, rows 64:127 the kappa'[1024,2048) chunk
    (disjoint row-groups run in parallel -> 2 cols/cycle).
  - PV is likewise row-tiled: each 128-kappa chunk contracts as two
    concurrent K=64 matmuls into separate PSUM accumulators Oa/Ob
    (summed during extraction).  V carries an all-ones 65th column so
    softmax denominators ride along.
  - O is normalized by 1/den BEFORE the merge (reciprocal_approx_fast +
    gpsimd partition_broadcast + DVE multiply), which lets the two
    heads' merges pack into single K=128 matmuls (lhsT rows 0:64 =
    head t=0, rows 64:128 = t=1) accumulating directly in PSUM; output
    rows DMA straight from PSUM to DRAM.  Phase order (t,H) =
    (0,0),(1,0),(0,1),(1,1) so both heads of an H-half finish together.
  - All matmuls bf16 (psum f32); probs bf16; exp on ScalarE (scale=1/8).
"""

import numpy as np

HIDDEN = 512
DHEAD = 64
B = 2
S = 2048
NCORES = 8

_CACHE = {}

LAST_EXEC_NS = None
LAST_RESULTS = None


def _build_nc():
    if "nc" in _CACHE:
        return _CACHE["nc"]

    import contextlib
    from collections import deque

    import concourse.mybir as mybir
    import concourse.tile as tile
    from concourse import bacc

    f32 = mybir.dt.float32
    bf16 = mybir.dt.bfloat16
    Exp = mybir.ActivationFunctionType.Exp
    Copy = mybir.ActivationFunctionType.Copy
    mult = mybir.AluOpType.mult
    add = mybir.AluOpType.add

    nc = bacc.Bacc("TRN2", target_bir_lowering=False)

    # ---- DRAM I/O ----
    d_x = {}
    d_w = {}
    for nm in ("q", "k", "v"):
        # pre-transposed on host to the SBUF layout -> contiguous DMA
        d_x[nm] = nc.dram_tensor(f"x{nm}T", [128, 2048], bf16, kind="ExternalInput")
        d_w[nm] = nc.dram_tensor(f"w{nm}T", [128, 2048], bf16, kind="ExternalInput")
    d_wm = nc.dram_tensor("wmP", [128, 512], bf16, kind="ExternalInput")
    d_bq = nc.dram_tensor("bq8", [128, 8], f32, kind="ExternalInput")
    d_bk = nc.dram_tensor("bk4", [128, 4], f32, kind="ExternalInput")
    d_bv = nc.dram_tensor("bv1", [1, 512], bf16, kind="ExternalInput")
    d_on = nc.dram_tensor("ones1", [1, 128], bf16, kind="ExternalInput")
    d_out = nc.dram_tensor("outp", [2048, 512], f32, kind="ExternalOutput")
    # row s' = 8*j + w  (j = q' mod 256, w = q' // 256)
    d_out_r = d_out.rearrange("(j w) e -> j w e", w=8)

    with tile.TileContext(nc) as tc:
        ctx = contextlib.ExitStack()
        with ctx:
            big = ctx.enter_context(tc.tile_pool(name="big", bufs=8))
            sp = ctx.enter_context(tc.tile_pool(name="sp", bufs=2, space="PSUM"))
            op = ctx.enter_context(tc.tile_pool(name="op", bufs=2, space="PSUM"))
            pp = ctx.enter_context(tc.tile_pool(name="pp", bufs=2, space="PSUM"))
            ptp = ctx.enter_context(tc.tile_pool(name="ptp", bufs=6))
            small = ctx.enter_context(tc.tile_pool(name="small", bufs=1))
            otp = ctx.enter_context(tc.tile_pool(name="otp", bufs=2))
            drp = ctx.enter_context(tc.tile_pool(name="drp", bufs=2))
            rcp = ctx.enter_context(tc.tile_pool(name="rcp", bufs=2))
            osp = ctx.enter_context(tc.tile_pool(name="osp", bufs=4))
            msp = ctx.enter_context(tc.tile_pool(name="msp", bufs=2))

            # ---- tiny warm-up constant ----
            ones_tiny = small.tile([1, 1], f32, tag="ones_tiny")
            nc.vector.memset(ones_tiny, 1.0)
            warm = small.tile([1, 1], f32, tag="warm")
            nc.scalar.activation(warm, ones_tiny, Exp, scale=1.0)

            bq8 = small.tile([128, 8], f32, tag="bq8")
            nc.gpsimd.dma_start(out=bq8, in_=d_bq[:, :])

            xin = {}
            win = {}
            for nm, eng in (("q", nc.sync), ("k", nc.scalar), ("v", nc.gpsimd)):
                xt = big.tile([128, 4, 512], bf16, tag="big", name=f"x{nm}")
                eng.dma_start(out=xt, in_=d_x[nm].rearrange("p (t d) -> p t d", t=4))
                wt = big.tile([128, 4, 512], bf16, tag="big", name=f"w{nm}")
                eng.dma_start(out=wt, in_=d_w[nm].rearrange("p (t d) -> p t d", t=4))
                xin[nm] = xt
                win[nm] = wt

            bk4 = small.tile([128, 4], f32, tag="bk4")
            nc.gpsimd.dma_start(out=bk4, in_=d_bk[:, :])
            bv1 = small.tile([1, 512], bf16, tag="bv1")
            nc.gpsimd.dma_start(out=bv1, in_=d_bv[:, :])
            ones1 = small.tile([1, 128], bf16, tag="ones1")
            nc.gpsimd.dma_start(out=ones1, in_=d_on[:, :])
            wm_sb = small.tile([128, 512], bf16, tag="wm", name="wm")
            nc.gpsimd.dma_start(out=wm_sb, in_=d_wm[:, :])

            # ---- persistent per-head tensors ----
            # QQ: rows 0:64 AND 64:128 both hold Q^T (dup for row tiling)
            QQ = small.tile([128, 2, 2048], bf16, tag="QQ", name="QQ")
            # KT2 rows 0:64 = K^T kappa' [0,1024); rows 64:128 = [1024,2048)
            KT2 = small.tile([128, 2, 1024], bf16, tag="KT2", name="KT2")
            # Vt[t][p, c, hv, f]: chunk jj=(2c+hv); f 0:64 = V, f 64 = 1.0
            Vt = [
                small.tile([128, 8, 2, 65], bf16, tag=f"V{t}", name=f"V{t}")
                for t in range(2)
            ]
            for t in range(2):
                nc.vector.memset(Vt[t][:, :, :, 64:65], 1.0)
            # OT[H]: rows 0:64 = normalized O^T of head t=0, 64:128 = t=1
            OT = [
                otp.tile([128, 1024], bf16, tag=f"OT{i}", name=f"OT{i}")
                for i in range(2)
            ]

            # ---- projection emitters ----
            def proj_q(i):
                # ps rows 0:64 = e-chunk 2i, rows 64:128 = e-chunk 2i+1;
                # each written to BOTH partition halves of QQ (row-tile dup).
                xt, wt = xin["q"], win["q"]
                ps = pp.tile([128, 512], f32, tag="pp", name="psq")
                for D in range(4):
                    nc.tensor.matmul(
                        ps,
                        wt[:, D, 128 * i : 128 * i + 128],
                        xt[:, D, :],
                        start=(D == 0),
                        stop=(D == 3),
                    )
                psr = ps.rearrange("p (t s) -> p t s", t=2)
                for half in range(2):
                    c = 2 * i + half
                    src = psr[64 * half : 64 * half + 64, :, :]
                    bias = bq8[64 * half : 64 * half + 64, c : c + 1]
                    nc.vector.tensor_scalar_add(
                        QQ[0:64, :, 256 * c : 256 * c + 256], src, bias
                    )
                    nc.vector.tensor_scalar_add(
                        QQ[64:128, :, 256 * c : 256 * c + 256], src, bias
                    )

            def proj_k(cp):
                xt, wt = xin["k"], win["k"]
                ps = pp.tile([128, 512], f32, tag="pp", name="psk")
                for D in range(4):
                    nc.tensor.matmul(
                        ps,
                        wt[:, D, 128 * cp : 128 * cp + 128],
                        xt[:, D, :],
                        start=(D == 0),
                        stop=(D == 3),
                    )
                nc.vector.tensor_scalar_add(
                    KT2[:, :, 256 * cp : 256 * cp + 256],
                    ps.rearrange("p (t s) -> p t s", t=2),
                    bk4[:, cp : cp + 1],
                )

            def proj_v(St):
                xt, wt = xin["v"], win["v"]
                ps = pp.tile([128, 512], f32, tag="pp", name="psv")
                for D in range(4):
                    nc.tensor.matmul(
                        ps,
                        xt[:, D, 128 * St : 128 * St + 128],
                        wt[:, D, :],
                        start=(D == 0),
                        stop=False,
                    )
                nc.tensor.matmul(
                    ps, ones1[0:1, :], bv1[0:1, :], start=False, stop=True
                )
                t, hv = St // 2, St % 2
                nc.vector.tensor_copy(
                    Vt[t][:, :, hv, 0:64],
                    ps.rearrange("p (c d) -> p c d", c=8),
                )

            # Upfront projections double as the HAM warm-up: 24 back-to-back
            # matmuls from DMA-land keep the PE busy through the cold window.
            proj_todo = deque()
            proj_q(0)
            proj_q(1)
            proj_k(0)
            proj_k(1)
            proj_v(0)
            proj_v(1)
            proj_todo.append(lambda: proj_k(2))
            proj_todo.append(lambda: proj_k(3))
            proj_todo.append(lambda: proj_q(2))
            proj_todo.append(lambda: proj_q(3))
            proj_todo.append(lambda: proj_v(2))
            proj_todo.append(lambda: proj_v(3))

            # ---- attention: 4 phases x 16 slots; slot = (n, jp) n-major ----
            phases = [(0, 0), (1, 0), (0, 1), (1, 1)]
            LAG = 2
            pend = deque()
            extract_pending = deque()
            merge_q = deque()
            O_tiles = {}

            def do_merge(H, ii, on_act=False):
                mp = pp.tile([128, 512], f32, tag="pp", name="mp")
                nc.tensor.matmul(mp, OT[H][:, 128 * ii : 128 * ii + 128], wm_sb)
                ms = msp.tile([128, 512], f32, tag="ms", name="ms")
                if on_act:
                    nc.scalar.activation(ms, mp, Copy, scale=1.0)
                else:
                    nc.vector.tensor_copy(ms, mp)
                j0 = 128 * (ii % 2)
                nc.sync.dma_start(
                    out=d_out_r[j0 : j0 + 128, 4 * H + ii // 2, :], in_=ms
                )

            def do_extract(pi, n):
                t, H = phases[pi]
                Oa, Ob = O_tiles[(pi, n)]
                ca = osp.tile([65, 512], f32, tag="ca", name="ca")
                nc.vector.tensor_copy(ca, Oa[0:65, :])
                dsum = drp.tile([1, 512], f32, tag="ds", name="dsum")
                nc.vector.tensor_tensor(dsum, Ob[64:65, :], ca[64:65, :], add)
                rr = drp.tile([1, 512], f32, tag="rr", name="rr")
                nc.vector.reciprocal_approx_fast(rr, dsum)
                rb = rcp.tile([64, 512], f32, tag="rb", name="rb")
                nc.gpsimd.partition_broadcast(rb, rr)
                osum = osp.tile([64, 512], f32, tag="os", name="osum")
                nc.vector.tensor_tensor(osum, Ob[0:64, :], ca[0:64, :], add)
                ots = OT[H][64 * t : 64 * t + 64, 512 * n : 512 * n + 512]
                nc.vector.tensor_tensor(ots, osum, rb, mult)
                if t == 1:
                    last = pi == 3 and n == 1
                    for ii in range(4 * n, 4 * n + 4):
                        merge_q.append(
                            lambda ii=ii, H=H, a=last: do_merge(H, ii, on_act=a)
                        )

            def issue_pv(pi, n, jp, pt):
                t, H = phases[pi]
                if (pi, n) not in O_tiles:
                    O_tiles[(pi, n)] = (
                        op.tile([65, 512], f32, tag="op", name="Oa"),
                        op.tile([65, 512], f32, tag="op", name="Ob"),
                    )
                Oa, Ob = O_tiles[(pi, n)]
                for ci in (0, 1):
                    jj = jp + 8 * ci
                    c, hv = jj // 2, jj % 2
                    st = jp == 0 and ci == 0
                    sp_ = jp == 7 and ci == 1
                    nc.tensor.matmul(
                        Oa[0:65, :],
                        Vt[t][0:64, c, hv, :],
                        pt[0:64, 512 * ci : 512 * ci + 512],
                        start=st,
                        stop=sp_,
                    )
                    nc.tensor.matmul(
                        Ob[0:65, :],
                        Vt[t][64:128, c, hv, :],
                        pt[64:128, 512 * ci : 512 * ci + 512],
                        start=st,
                        stop=sp_,
                    )
                if jp == 7:
                    extract_pending.append((pi, n))

            for pi, (t, H) in enumerate(phases):
                for s in range(16):
                    n, jp = s // 8, s % 8
                    if proj_todo:
                        proj_todo.popleft()()
                    sT = sp.tile([128, 1024], f32, tag="sp", name="sT")
                    nc.tensor.matmul(
                        sT[:, 0:512],
                        KT2[0:64, t, 128 * jp : 128 * jp + 128],
                        QQ[0:64, t, 1024 * H + 512 * n : 1024 * H + 512 * n + 512],
                    )
                    nc.tensor.matmul(
                        sT[:, 512:1024],
                        KT2[64:128, t, 128 * jp : 128 * jp + 128],
                        QQ[64:128, t, 1024 * H + 512 * n : 1024 * H + 512 * n + 512],
                    )
                    pt = ptp.tile([128, 1024], bf16, tag="pt")
                    nc.scalar.activation(pt, sT, Exp, scale=0.125)
                    while extract_pending:
                        do_extract(*extract_pending.popleft())
                    pend.append((pi, n, jp, pt))
                    if pend and pend[0][0] < pi:
                        issue_pv(*pend.popleft())
                    if len(pend) > LAG:
                        issue_pv(*pend.popleft())
                    if merge_q and s < 14:
                        merge_q.popleft()()
                    if pi == 3 and merge_q and s < 14:
                        merge_q.popleft()()
            while pend:
                issue_pv(*pend.popleft())
                while extract_pending:
                    do_extract(*extract_pending.popleft())
            while extract_pending:
                do_extract(*extract_pending.popleft())
            while merge_q:
                merge_q.popleft()()

    nc.finalize()
    _CACHE["nc"] = nc
    return nc


def _prep_in_maps(q, k, v, Wq, Wk, Wv, Wm, bq, bk, bv):
    import ml_dtypes

    f = np.float32
    b16 = ml_dtypes.bfloat16

    def cont(a):
        # [512, C] -> [128, 4*C]: row t*128+p -> partition p, block t
        C = a.shape[1]
        return np.ascontiguousarray(
            a.reshape(4, 128, C).transpose(1, 0, 2).reshape(128, 4 * C)
        )

    WqT = Wq.T.astype(b16)  # plain [512, 512]
    # wkT: columns paired (c, c+4) -> col 128*cp + 64*h + d = orig (cp+4h)*64+d
    WkT = Wk.T.astype(b16)
    WkT = np.ascontiguousarray(
        WkT.reshape(512, 2, 4, 64).transpose(0, 2, 1, 3).reshape(512, 512)
    )
    WvT = np.ascontiguousarray(Wv.T.astype(b16))
    WmT = Wm.T.astype(f)  # [e_in, e_out]

    t_ = np.ascontiguousarray(bq.astype(f).reshape(8, 64).T)  # [64, 8]
    bq8 = np.ascontiguousarray(np.vstack([t_, t_]))  # [128, 8]
    kk = bk.astype(f).reshape(8, 64)
    bk4 = np.ascontiguousarray(np.concatenate([kk[0:4].T, kk[4:8].T], axis=0))
    bv1 = np.ascontiguousarray(bv.astype(b16).reshape(1, 512))

    in_maps = []
    for c in range(NCORES):
        b_, j4 = c // 4, c % 4
        r0 = 512 * j4
        m = {
            "xqT": cont(q[b_, r0 : r0 + 512, :].T.astype(b16)),
            "xkT": cont(k[b_, r0 : r0 + 512, :].T.astype(b16)),
            "xvT": cont(v[b_, r0 : r0 + 512, :].T.astype(b16)),
            "wqT": cont(WqT),
            "wkT": cont(WkT),
            "wvT": cont(WvT),
            "wmP": np.ascontiguousarray(
                WmT[128 * j4 : 128 * j4 + 128, :].astype(b16)
            ),
            "bq8": bq8,
            "bk4": bk4,
            "bv1": bv1,
            "ones1": np.ones((1, 128), b16),
        }
        in_maps.append(m)
    return in_maps


def _reference_fallback(q, k, v, mask, Wq, Wk, Wv, Wm, bq, bk, bv, bm):
    # Only used if mask is nonzero (spec fills it with zeros).
    f = np.float32
    qh = (q.astype(f) @ Wq.T.astype(f) + bq).reshape(B, 8, S, DHEAD)
    kh = (k.astype(f) @ Wk.T.astype(f) + bk).reshape(B, 8, S, DHEAD)
    vh = (v.astype(f) @ Wv.T.astype(f) + bv).reshape(B, 8, S, DHEAD)
    s = np.einsum("bhqd,bhkd->bhqk", qh, kh) / np.sqrt(np.float32(DHEAD))
    s = np.where(mask, np.float32(-1e9), s)
    s = s - s.max(-1, keepdims=True)
    e = np.exp(s)
    p = e / e.sum(-1, keepdims=True)
    attn = np.einsum("bhqk,bhkd->bhqd", p, vh)
    attn = attn.transpose(0, 2, 1, 3).reshape(B, S, HIDDEN)
    return attn @ Wm.T.astype(f) + bm


def kernel(q, k, v, mask, Wq, Wk, Wv, Wm, bq, bk, bv, bm):
    global LAST_EXEC_NS, LAST_RESULTS
    q, k, v = (np.asarray(a, np.float32) for a in (q, k, v))
    mask = np.asarray(mask)
    Wq, Wk, Wv, Wm = (np.asarray(a, np.float32) for a in (Wq, Wk, Wv, Wm))
    bq, bk, bv, bm = (np.asarray(a, np.float32) for a in (bq, bk, bv, bm))

    if mask.any():
        return _reference_fallback(q, k, v, mask, Wq, Wk, Wv, Wm, bq, bk, bv, bm)

    from concourse.bass_utils import run_bass_kernel_spmd

    nc = _build_nc()
    in_maps = _prep_in_maps(q, k, v, Wq, Wk, Wv, Wm, bq, bk, bv)
    res = run_bass_kernel_spmd(nc, in_maps, list(range(NCORES)))
    LAST_RESULTS = res
    LAST_EXEC_NS = getattr(res, "exec_time_ns", None)

    out = np.zeros((B, S, HIDDEN), np.float32)
    for c in range(NCORES):
        out[c // 4] += res.results[c]["outp"]
    out += bm
    return out


# revision 35
# speedup vs baseline: 1.0582x; 1.0497x over previous
"""Trainium2 Bass kernel for nn_MultiHeadAttention_46188078301212.

Module semantics (replicated from the PyTorch module's quirky reshape):
  P_q = q @ Wq.T + bq  (same for k, v), each [B, 2048, 512]
  Head h takes projection rows [256h, 256h+256) viewed as [2048, 64]
  (row-major), runs standard softmax attention, heads are concatenated
  along hidden (out col block 64h..64h+64) and merged with Wm.T + bm.

Sharding: 8 cores = (batch b in {0,1}) x (head-pair j4 in {0..3}).
Core c handles b = c//4 and global heads {2*j4, 2*j4+1}; each core
emits a [2048, 512] partial (its 2 heads' contribution); host sums 4
partials per batch and adds bm.

Key on-core structure (v2 — row-tiled):
  - Working index q' = 256*(e//64) + s is a permutation of the true
    sequence index s' = 8*s + e//64; the final DMA un-permutes.
  - Scores are computed transposed (kappa on partitions, q on free) as
    CONCURRENT K=64 row-tiled matmul pairs: PE rows 0:63 compute the
    kappa'[0,1024) chunk, rows 64:127 the kappa'[1024,2048) chunk
    (disjoint row-groups run in parallel -> 2 cols/cycle).
  - PV is likewise row-tiled: each 128-kappa chunk contracts as two
    concurrent K=64 matmuls into separate PSUM accumulators Oa/Ob
    (summed during extraction).  V carries an all-ones 65th column so
    softmax denominators ride along.
  - O is normalized by 1/den BEFORE the merge (reciprocal_approx_fast +
    gpsimd partition_broadcast + DVE multiply), which lets the two
    heads' merges pack into single K=128 matmuls (lhsT rows 0:64 =
    head t=0, rows 64:128 = t=1) accumulating directly in PSUM; output
    rows DMA straight from PSUM to DRAM.  Phase order (t,H) =
    (0,0),(1,0),(0,1),(1,1) so both heads of an H-half finish together.
  - All matmuls bf16 (psum f32); probs bf16; exp on ScalarE (scale=1/8).
"""

import numpy as np

HIDDEN = 512
DHEAD = 64
B = 2
S = 2048
NCORES = 8

_CACHE = {}

LAST_EXEC_NS = None
LAST_RESULTS = None


def _build_nc():
    if "nc" in _CACHE:
        return _CACHE["nc"]

    import contextlib
    from collections import deque

    import concourse.mybir as mybir
    import concourse.tile as tile
    from concourse import bacc

    f32 = mybir.dt.float32
    bf16 = mybir.dt.bfloat16
    Exp = mybir.ActivationFunctionType.Exp
    Copy = mybir.ActivationFunctionType.Copy
    Identity = mybir.ActivationFunctionType.Identity
    mult = mybir.AluOpType.mult
    add = mybir.AluOpType.add

    nc = bacc.Bacc("TRN2", target_bir_lowering=False)

    # ---- DRAM I/O ----
    d_x = {}
    d_w = {}
    for nm in ("q", "k", "v"):
        # pre-transposed on host to the SBUF layout -> contiguous DMA
        d_x[nm] = nc.dram_tensor(f"x{nm}T", [128, 2048], bf16, kind="ExternalInput")
        d_w[nm] = nc.dram_tensor(f"w{nm}T", [128, 2048], bf16, kind="ExternalInput")
    d_wm = nc.dram_tensor("wmP", [128, 512], bf16, kind="ExternalInput")
    d_bq = nc.dram_tensor("bq8", [128, 8], f32, kind="ExternalInput")
    d_bk = nc.dram_tensor("bk4", [128, 4], f32, kind="ExternalInput")
    d_bv = nc.dram_tensor("bv1", [1, 512], bf16, kind="ExternalInput")
    d_on = nc.dram_tensor("ones1", [1, 128], bf16, kind="ExternalInput")
    d_out = nc.dram_tensor("outp", [2048, 512], f32, kind="ExternalOutput")
    # row s' = 8*j + w  (j = q' mod 256, w = q' // 256)
    d_out_r = d_out.rearrange("(j w) e -> j w e", w=8)

    with tile.TileContext(nc) as tc:
        ctx = contextlib.ExitStack()
        with ctx:
            big = ctx.enter_context(tc.tile_pool(name="big", bufs=8))
            sp = ctx.enter_context(tc.tile_pool(name="sp", bufs=2, space="PSUM"))
            op = ctx.enter_context(tc.tile_pool(name="op", bufs=2, space="PSUM"))
            pp = ctx.enter_context(tc.tile_pool(name="pp", bufs=2, space="PSUM"))
            ptp = ctx.enter_context(tc.tile_pool(name="ptp", bufs=6))
            small = ctx.enter_context(tc.tile_pool(name="small", bufs=1))
            otp = ctx.enter_context(tc.tile_pool(name="otp", bufs=2))
            drp = ctx.enter_context(tc.tile_pool(name="drp", bufs=2))
            rcp = ctx.enter_context(tc.tile_pool(name="rcp", bufs=2))
            osp = ctx.enter_context(tc.tile_pool(name="osp", bufs=4))
            msp = ctx.enter_context(tc.tile_pool(name="msp", bufs=2))

            # ---- tiny warm-up constant ----
            ones_tiny = small.tile([1, 1], f32, tag="ones_tiny")
            nc.vector.memset(ones_tiny, 1.0)
            warm = small.tile([1, 1], f32, tag="warm")
            nc.scalar.activation(warm, ones_tiny, Exp, scale=1.0)

            bq8 = small.tile([128, 8], f32, tag="bq8")
            nc.gpsimd.dma_start(out=bq8, in_=d_bq[:, :])

            xin = {}
            win = {}
            for nm, eng in (("q", nc.sync), ("k", nc.scalar), ("v", nc.gpsimd)):
                xt = big.tile([128, 4, 512], bf16, tag="big", name=f"x{nm}")
                eng.dma_start(out=xt, in_=d_x[nm].rearrange("p (t d) -> p t d", t=4))
                wt = big.tile([128, 4, 512], bf16, tag="big", name=f"w{nm}")
                eng.dma_start(out=wt, in_=d_w[nm].rearrange("p (t d) -> p t d", t=4))
                xin[nm] = xt
                win[nm] = wt

            bk4 = small.tile([128, 4], f32, tag="bk4")
            nc.gpsimd.dma_start(out=bk4, in_=d_bk[:, :])
            bv1 = small.tile([1, 512], bf16, tag="bv1")
            nc.gpsimd.dma_start(out=bv1, in_=d_bv[:, :])
            ones1 = small.tile([1, 128], bf16, tag="ones1")
            nc.gpsimd.dma_start(out=ones1, in_=d_on[:, :])
            wm_sb = small.tile([128, 512], bf16, tag="wm", name="wm")
            nc.gpsimd.dma_start(out=wm_sb, in_=d_wm[:, :])

            # ---- persistent per-head tensors ----
            # QQ: rows 0:64 AND 64:128 both hold Q^T (dup for row tiling)
            QQ = small.tile([128, 2, 2048], bf16, tag="QQ", name="QQ")
            # KT2 rows 0:64 = K^T kappa' [0,1024); rows 64:128 = [1024,2048)
            KT2 = small.tile([128, 2, 1024], bf16, tag="KT2", name="KT2")
            # Vt[t][p, c, hv, f]: chunk jj=(2c+hv); f 0:64 = V, f 64 = 1.0
            Vt = [
                small.tile([128, 8, 2, 65], bf16, tag=f"V{t}", name=f"V{t}")
                for t in range(2)
            ]
            for t in range(2):
                nc.vector.memset(Vt[t][:, :, :, 64:65], 1.0)
            # OT[H]: rows 0:64 = normalized O^T of head t=0, 64:128 = t=1
            OT = [
                otp.tile([128, 1024], bf16, tag=f"OT{i}", name=f"OT{i}")
                for i in range(2)
            ]

            # ---- projection emitters ----
            def proj_q(i):
                # ps rows 0:64 = e-chunk 2i, rows 64:128 = e-chunk 2i+1;
                # each written to BOTH partition halves of QQ (row-tile dup).
                xt, wt = xin["q"], win["q"]
                ps = pp.tile([128, 512], f32, tag="pp", name="psq")
                for D in range(4):
                    nc.tensor.matmul(
                        ps,
                        wt[:, D, 128 * i : 128 * i + 128],
                        xt[:, D, :],
                        start=(D == 0),
                        stop=(D == 3),
                    )
                psr = ps.rearrange("p (t s) -> p t s", t=2)
                for half in range(2):
                    c = 2 * i + half
                    src = psr[64 * half : 64 * half + 64, :, :]
                    bias = bq8[64 * half : 64 * half + 64, c : c + 1]
                    nc.vector.tensor_scalar_add(
                        QQ[0:64, :, 256 * c : 256 * c + 256], src, bias
                    )
                    # duplicate row-tile copy via the (otherwise idle) ACT
                    nc.scalar.activation(
                        QQ[64:128, :, 256 * c : 256 * c + 256],
                        src,
                        Identity,
                        bias=bias,
                    )

            def proj_k(cp):
                xt, wt = xin["k"], win["k"]
                ps = pp.tile([128, 512], f32, tag="pp", name="psk")
                for D in range(4):
                    nc.tensor.matmul(
                        ps,
                        wt[:, D, 128 * cp : 128 * cp + 128],
                        xt[:, D, :],
                        start=(D == 0),
                        stop=(D == 3),
                    )
                nc.vector.tensor_scalar_add(
                    KT2[:, :, 256 * cp : 256 * cp + 256],
                    ps.rearrange("p (t s) -> p t s", t=2),
                    bk4[:, cp : cp + 1],
                )

            def proj_v(St):
                xt, wt = xin["v"], win["v"]
                ps = pp.tile([128, 512], f32, tag="pp", name="psv")
                for D in range(4):
                    nc.tensor.matmul(
                        ps,
                        xt[:, D, 128 * St : 128 * St + 128],
                        wt[:, D, :],
                        start=(D == 0),
                        stop=False,
                    )
                nc.tensor.matmul(
                    ps, ones1[0:1, :], bv1[0:1, :], start=False, stop=True
                )
                t, hv = St // 2, St % 2
                nc.vector.tensor_copy(
                    Vt[t][:, :, hv, 0:64],
                    ps.rearrange("p (c d) -> p c d", c=8),
                )

            # Upfront projections double as the HAM warm-up: 24 back-to-back
            # matmuls from DMA-land keep the PE busy through the cold window.
            proj_todo = deque()
            proj_q(0)
            proj_k(0)
            proj_q(1)
            proj_k(1)
            proj_v(0)
            proj_v(1)
            proj_todo.append(lambda: proj_k(2))
            proj_todo.append(lambda: proj_k(3))
            proj_todo.append(lambda: proj_q(2))
            proj_todo.append(lambda: proj_q(3))
            proj_todo.append(lambda: proj_v(2))
            proj_todo.append(lambda: proj_v(3))

            # ---- attention: 4 phases x 16 slots; slot = (n, jp) n-major ----
            phases = [(0, 0), (1, 0), (0, 1), (1, 1)]
            LAG = 2
            pend = deque()
            extract_pending = deque()
            merge_q = deque()
            O_tiles = {}

            def do_merge(H, ii, on_act=False):
                mp = pp.tile([128, 512], f32, tag="pp", name="mp")
                nc.tensor.matmul(mp, OT[H][:, 128 * ii : 128 * ii + 128], wm_sb)
                ms = msp.tile([128, 512], f32, tag="ms", name="ms")
                if on_act:
                    nc.scalar.activation(ms, mp, Copy, scale=1.0)
                else:
                    nc.vector.tensor_copy(ms, mp)
                j0 = 128 * (ii % 2)
                nc.sync.dma_start(
                    out=d_out_r[j0 : j0 + 128, 4 * H + ii // 2, :], in_=ms
                )

            def do_extract(pi, n):
                t, H = phases[pi]
                Oa, Ob = O_tiles[(pi, n)]
                ca = osp.tile([65, 512], f32, tag="ca", name="ca")
                nc.vector.tensor_copy(ca, Oa[0:65, :])
                dsum = drp.tile([1, 512], f32, tag="ds", name="dsum")
                nc.vector.tensor_tensor(dsum, Ob[64:65, :], ca[64:65, :], add)
                rr = drp.tile([1, 512], f32, tag="rr", name="rr")
                nc.vector.reciprocal_approx_fast(rr, dsum)
                rb = rcp.tile([64, 512], f32, tag="rb", name="rb")
                nc.gpsimd.partition_broadcast(rb, rr)
                osum = osp.tile([64, 512], f32, tag="os", name="osum")
                nc.vector.tensor_tensor(osum, Ob[0:64, :], ca[0:64, :], add)
                ots = OT[H][64 * t : 64 * t + 64, 512 * n : 512 * n + 512]
                nc.vector.tensor_tensor(ots, osum, rb, mult)
                if t == 1:
                    last = pi == 3 and n == 1
                    for ii in range(4 * n, 4 * n + 4):
                        merge_q.append(
                            lambda ii=ii, H=H, a=last: do_merge(H, ii, on_act=a)
                        )

            def issue_pv(pi, n, jp, pt):
                t, H = phases[pi]
                if (pi, n) not in O_tiles:
                    O_tiles[(pi, n)] = (
                        op.tile([65, 512], f32, tag="op", name="Oa"),
                        op.tile([65, 512], f32, tag="op", name="Ob"),
                    )
                Oa, Ob = O_tiles[(pi, n)]
                for ci in (0, 1):
                    jj = jp + 8 * ci
                    c, hv = jj // 2, jj % 2
                    st = jp == 0 and ci == 0
                    sp_ = jp == 7 and ci == 1
                    nc.tensor.matmul(
                        Oa[0:65, :],
                        Vt[t][0:64, c, hv, :],
                        pt[0:64, 512 * ci : 512 * ci + 512],
                        start=st,
                        stop=sp_,
                    )
                    nc.tensor.matmul(
                        Ob[0:65, :],
                        Vt[t][64:128, c, hv, :],
                        pt[64:128, 512 * ci : 512 * ci + 512],
                        start=st,
                        stop=sp_,
                    )
                if jp == 7:
                    extract_pending.append((pi, n))

            for pi, (t, H) in enumerate(phases):
                for s in range(16):
                    n, jp = s // 8, s % 8
                    if proj_todo and s % 2 == 0:
                        proj_todo.popleft()()
                    sT = sp.tile([128, 1024], f32, tag="sp", name="sT")
                    nc.tensor.matmul(
                        sT[:, 0:512],
                        KT2[0:64, t, 128 * jp : 128 * jp + 128],
                        QQ[0:64, t, 1024 * H + 512 * n : 1024 * H + 512 * n + 512],
                    )
                    nc.tensor.matmul(
                        sT[:, 512:1024],
                        KT2[64:128, t, 128 * jp : 128 * jp + 128],
                        QQ[64:128, t, 1024 * H + 512 * n : 1024 * H + 512 * n + 512],
                    )
                    pt = ptp.tile([128, 1024], bf16, tag="pt")
                    nc.scalar.activation(pt, sT, Exp, scale=0.125)
                    while extract_pending:
                        do_extract(*extract_pending.popleft())
                    pend.append((pi, n, jp, pt))
                    if pend and pend[0][0] < pi:
                        issue_pv(*pend.popleft())
                    if len(pend) > LAG:
                        issue_pv(*pend.popleft())
                    if merge_q and (pi == 3 or s < 14):
                        merge_q.popleft()()
                    if pi == 3 and merge_q:
                        merge_q.popleft()()
            while pend:
                issue_pv(*pend.popleft())
                while extract_pending:
                    do_extract(*extract_pending.popleft())
            while extract_pending:
                do_extract(*extract_pending.popleft())
            while merge_q:
                merge_q.popleft()()

    nc.finalize()
    _CACHE["nc"] = nc
    return nc


def _prep_in_maps(q, k, v, Wq, Wk, Wv, Wm, bq, bk, bv):
    import ml_dtypes

    f = np.float32
    b16 = ml_dtypes.bfloat16

    def cont(a):
        # [512, C] -> [128, 4*C]: row t*128+p -> partition p, block t
        C = a.shape[1]
        return np.ascontiguousarray(
            a.reshape(4, 128, C).transpose(1, 0, 2).reshape(128, 4 * C)
        )

    WqT = Wq.T.astype(b16)  # plain [512, 512]
    # wkT: columns paired (c, c+4) -> col 128*cp + 64*h + d = orig (cp+4h)*64+d
    WkT = Wk.T.astype(b16)
    WkT = np.ascontiguousarray(
        WkT.reshape(512, 2, 4, 64).transpose(0, 2, 1, 3).reshape(512, 512)
    )
    WvT = np.ascontiguousarray(Wv.T.astype(b16))
    WmT = Wm.T.astype(f)  # [e_in, e_out]

    t_ = np.ascontiguousarray(bq.astype(f).reshape(8, 64).T)  # [64, 8]
    bq8 = np.ascontiguousarray(np.vstack([t_, t_]))  # [128, 8]
    kk = bk.astype(f).reshape(8, 64)
    bk4 = np.ascontiguousarray(np.concatenate([kk[0:4].T, kk[4:8].T], axis=0))
    bv1 = np.ascontiguousarray(bv.astype(b16).reshape(1, 512))

    in_maps = []
    for c in range(NCORES):
        b_, j4 = c // 4, c % 4
        r0 = 512 * j4
        m = {
            "xqT": cont(q[b_, r0 : r0 + 512, :].T.astype(b16)),
            "xkT": cont(k[b_, r0 : r0 + 512, :].T.astype(b16)),
            "xvT": cont(v[b_, r0 : r0 + 512, :].T.astype(b16)),
            "wqT": cont(WqT),
            "wkT": cont(WkT),
            "wvT": cont(WvT),
            "wmP": np.ascontiguousarray(
                WmT[128 * j4 : 128 * j4 + 128, :].astype(b16)
            ),
            "bq8": bq8,
            "bk4": bk4,
            "bv1": bv1,
            "ones1": np.ones((1, 128), b16),
        }
        in_maps.append(m)
    return in_maps


def _reference_fallback(q, k, v, mask, Wq, Wk, Wv, Wm, bq, bk, bv, bm):
    # Only used if mask is nonzero (spec fills it with zeros).
    f = np.float32
    qh = (q.astype(f) @ Wq.T.astype(f) + bq).reshape(B, 8, S, DHEAD)
    kh = (k.astype(f) @ Wk.T.astype(f) + bk).reshape(B, 8, S, DHEAD)
    vh = (v.astype(f) @ Wv.T.astype(f) + bv).reshape(B, 8, S, DHEAD)
    s = np.einsum("bhqd,bhkd->bhqk", qh, kh) / np.sqrt(np.float32(DHEAD))
    s = np.where(mask, np.float32(-1e9), s)
    s = s - s.max(-1, keepdims=True)
    e = np.exp(s)
    p = e / e.sum(-1, keepdims=True)
    attn = np.einsum("bhqk,bhkd->bhqd", p, vh)
    attn = attn.transpose(0, 2, 1, 3).reshape(B, S, HIDDEN)
    return attn @ Wm.T.astype(f) + bm


def kernel(q, k, v, mask, Wq, Wk, Wv, Wm, bq, bk, bv, bm):
    global LAST_EXEC_NS, LAST_RESULTS
    q, k, v = (np.asarray(a, np.float32) for a in (q, k, v))
    mask = np.asarray(mask)
    Wq, Wk, Wv, Wm = (np.asarray(a, np.float32) for a in (Wq, Wk, Wv, Wm))
    bq, bk, bv, bm = (np.asarray(a, np.float32) for a in (bq, bk, bv, bm))

    if mask.any():
        return _reference_fallback(q, k, v, mask, Wq, Wk, Wv, Wm, bq, bk, bv, bm)

    from concourse.bass_utils import run_bass_kernel_spmd

    nc = _build_nc()
    in_maps = _prep_in_maps(q, k, v, Wq, Wk, Wv, Wm, bq, bk, bv)
    res = run_bass_kernel_spmd(nc, in_maps, list(range(NCORES)))
    LAST_RESULTS = res
    LAST_EXEC_NS = getattr(res, "exec_time_ns", None)

    out = np.zeros((B, S, HIDDEN), np.float32)
    for c in range(NCORES):
        out[c // 4] += res.results[c]["outp"]
    out += bm
    return out


# revision 42
# speedup vs baseline: 1.0647x; 1.0061x over previous
"""Trainium2 Bass kernel for nn_MultiHeadAttention_46188078301212.

Module semantics (replicated from the PyTorch module's quirky reshape):
  P_q = q @ Wq.T + bq  (same for k, v), each [B, 2048, 512]
  Head h takes projection rows [256h, 256h+256) viewed as [2048, 64]
  (row-major), runs standard softmax attention, heads are concatenated
  along hidden (out col block 64h..64h+64) and merged with Wm.T + bm.

Sharding: 8 cores = (batch b in {0,1}) x (head-pair j4 in {0..3}).
Core c handles b = c//4 and global heads {2*j4, 2*j4+1}; each core
emits a [2048, 512] partial (its 2 heads' contribution); host sums 4
partials per batch and adds bm.

Key on-core structure (v2 — row-tiled):
  - Working index q' = 256*(e//64) + s is a permutation of the true
    sequence index s' = 8*s + e//64; the final DMA un-permutes.
  - Scores are computed transposed (kappa on partitions, q on free) as
    CONCURRENT K=64 row-tiled matmul pairs: PE rows 0:63 compute the
    kappa'[0,1024) chunk, rows 64:127 the kappa'[1024,2048) chunk
    (disjoint row-groups run in parallel -> 2 cols/cycle).
  - PV is likewise row-tiled: each 128-kappa chunk contracts as two
    concurrent K=64 matmuls into separate PSUM accumulators Oa/Ob
    (summed during extraction).  V carries an all-ones 65th column so
    softmax denominators ride along.
  - O is normalized by 1/den BEFORE the merge (reciprocal_approx_fast +
    gpsimd partition_broadcast + DVE multiply), which lets the two
    heads' merges pack into single K=128 matmuls (lhsT rows 0:64 =
    head t=0, rows 64:128 = t=1) accumulating directly in PSUM; output
    rows DMA straight from PSUM to DRAM.  Phase order (t,H) =
    (0,0),(1,0),(0,1),(1,1) so both heads of an H-half finish together.
  - All matmuls bf16 (psum f32); probs bf16; exp on ScalarE (scale=1/8).
"""

import numpy as np

HIDDEN = 512
DHEAD = 64
B = 2
S = 2048
NCORES = 8

_CACHE = {}

LAST_EXEC_NS = None
LAST_RESULTS = None


def _build_nc():
    if "nc" in _CACHE:
        return _CACHE["nc"]

    import contextlib
    from collections import deque

    import concourse.mybir as mybir
    import concourse.tile as tile
    from concourse import bacc

    f32 = mybir.dt.float32
    bf16 = mybir.dt.bfloat16
    Exp = mybir.ActivationFunctionType.Exp
    Copy = mybir.ActivationFunctionType.Copy
    Identity = mybir.ActivationFunctionType.Identity
    mult = mybir.AluOpType.mult
    add = mybir.AluOpType.add

    nc = bacc.Bacc("TRN2", target_bir_lowering=False)

    # ---- DRAM I/O ----
    d_x = {}
    d_w = {}
    for nm in ("q", "k", "v"):
        # pre-transposed on host to the SBUF layout -> contiguous DMA
        d_x[nm] = nc.dram_tensor(f"x{nm}T", [128, 2048], bf16, kind="ExternalInput")
        d_w[nm] = nc.dram_tensor(f"w{nm}T", [128, 2048], bf16, kind="ExternalInput")
    d_wm = nc.dram_tensor("wmP", [128, 512], bf16, kind="ExternalInput")
    d_bq = nc.dram_tensor("bq8", [128, 8], f32, kind="ExternalInput")
    d_bk = nc.dram_tensor("bk4", [128, 4], f32, kind="ExternalInput")
    d_bv = nc.dram_tensor("bv1", [1, 512], bf16, kind="ExternalInput")
    d_on = nc.dram_tensor("ones1", [1, 128], bf16, kind="ExternalInput")
    d_out = nc.dram_tensor("outp", [2048, 512], bf16, kind="ExternalOutput")
    # row s' = 8*j + w  (j = q' mod 256, w = q' // 256)
    d_out_r = d_out.rearrange("(j w) e -> j w e", w=8)

    with tile.TileContext(nc) as tc:
        ctx = contextlib.ExitStack()
        with ctx:
            big = ctx.enter_context(tc.tile_pool(name="big", bufs=8))
            sp = ctx.enter_context(tc.tile_pool(name="sp", bufs=2, space="PSUM"))
            op = ctx.enter_context(tc.tile_pool(name="op", bufs=2, space="PSUM"))
            pp = ctx.enter_context(tc.tile_pool(name="pp", bufs=2, space="PSUM"))
            ptp = ctx.enter_context(tc.tile_pool(name="ptp", bufs=6))
            small = ctx.enter_context(tc.tile_pool(name="small", bufs=1))
            otp = ctx.enter_context(tc.tile_pool(name="otp", bufs=2))
            drp = ctx.enter_context(tc.tile_pool(name="drp", bufs=2))
            rcp = ctx.enter_context(tc.tile_pool(name="rcp", bufs=2))
            osp = ctx.enter_context(tc.tile_pool(name="osp", bufs=4))
            msp = ctx.enter_context(tc.tile_pool(name="msp", bufs=2))

            # ---- tiny warm-up constant ----
            ones_tiny = small.tile([1, 1], f32, tag="ones_tiny")
            nc.vector.memset(ones_tiny, 1.0)
            warm = small.tile([1, 1], f32, tag="warm")
            nc.scalar.activation(warm, ones_tiny, Exp, scale=1.0)

            bq8 = small.tile([128, 8], f32, tag="bq8")
            nc.gpsimd.dma_start(out=bq8, in_=d_bq[:, :])

            xin = {}
            win = {}
            for nm, eng in (("q", nc.sync), ("k", nc.scalar), ("v", nc.gpsimd)):
                xt = big.tile([128, 4, 512], bf16, tag="big", name=f"x{nm}")
                eng.dma_start(out=xt, in_=d_x[nm].rearrange("p (t d) -> p t d", t=4))
                wt = big.tile([128, 4, 512], bf16, tag="big", name=f"w{nm}")
                eng.dma_start(out=wt, in_=d_w[nm].rearrange("p (t d) -> p t d", t=4))
                xin[nm] = xt
                win[nm] = wt

            bk4 = small.tile([128, 4], f32, tag="bk4")
            nc.gpsimd.dma_start(out=bk4, in_=d_bk[:, :])
            bv1 = small.tile([1, 512], bf16, tag="bv1")
            nc.gpsimd.dma_start(out=bv1, in_=d_bv[:, :])
            ones1 = small.tile([1, 128], bf16, tag="ones1")
            nc.gpsimd.dma_start(out=ones1, in_=d_on[:, :])
            wm_sb = small.tile([128, 512], bf16, tag="wm", name="wm")
            nc.gpsimd.dma_start(out=wm_sb, in_=d_wm[:, :])

            # ---- persistent per-head tensors ----
            # QQ: rows 0:64 AND 64:128 both hold Q^T (dup for row tiling)
            QQ = small.tile([128, 2, 2048], bf16, tag="QQ", name="QQ")
            # KT2 rows 0:64 = K^T kappa' [0,1024); rows 64:128 = [1024,2048)
            KT2 = small.tile([128, 2, 1024], bf16, tag="KT2", name="KT2")
            # Vt[t][p, c, hv, f]: chunk jj=(2c+hv); f 0:64 = V, f 64 = 1.0
            Vt = [
                small.tile([128, 8, 2, 65], bf16, tag=f"V{t}", name=f"V{t}")
                for t in range(2)
            ]
            for t in range(2):
                nc.vector.memset(Vt[t][:, :, :, 64:65], 1.0)
            # OT[H]: rows 0:64 = normalized O^T of head t=0, 64:128 = t=1
            OT = [
                otp.tile([128, 1024], bf16, tag=f"OT{i}", name=f"OT{i}")
                for i in range(2)
            ]

            # ---- projection emitters ----
            def proj_q(i):
                # ps rows 0:64 = e-chunk 2i, rows 64:128 = e-chunk 2i+1;
                # each written to BOTH partition halves of QQ (row-tile dup).
                xt, wt = xin["q"], win["q"]
                ps = pp.tile([128, 512], f32, tag="pp", name="psq")
                for D in range(4):
                    nc.tensor.matmul(
                        ps,
                        wt[:, D, 128 * i : 128 * i + 128],
                        xt[:, D, :],
                        start=(D == 0),
                        stop=(D == 3),
                    )
                psr = ps.rearrange("p (t s) -> p t s", t=2)
                for half in range(2):
                    c = 2 * i + half
                    src = psr[64 * half : 64 * half + 64, :, :]
                    bias = bq8[64 * half : 64 * half + 64, c : c + 1]
                    nc.vector.tensor_scalar_add(
                        QQ[0:64, :, 256 * c : 256 * c + 256], src, bias
                    )
                    # duplicate row-tile copy via the (otherwise idle) ACT
                    nc.scalar.activation(
                        QQ[64:128, :, 256 * c : 256 * c + 256],
                        src,
                        Identity,
                        bias=bias,
                    )

            def proj_k(cp):
                xt, wt = xin["k"], win["k"]
                ps = pp.tile([128, 512], f32, tag="pp", name="psk")
                for D in range(4):
                    nc.tensor.matmul(
                        ps,
                        wt[:, D, 128 * cp : 128 * cp + 128],
                        xt[:, D, :],
                        start=(D == 0),
                        stop=(D == 3),
                    )
                nc.vector.tensor_scalar_add(
                    KT2[:, :, 256 * cp : 256 * cp + 256],
                    ps.rearrange("p (t s) -> p t s", t=2),
                    bk4[:, cp : cp + 1],
                )

            def proj_v(St):
                xt, wt = xin["v"], win["v"]
                ps = pp.tile([128, 512], f32, tag="pp", name="psv")
                for D in range(4):
                    nc.tensor.matmul(
                        ps,
                        xt[:, D, 128 * St : 128 * St + 128],
                        wt[:, D, :],
                        start=(D == 0),
                        stop=False,
                    )
                nc.tensor.matmul(
                    ps, ones1[0:1, :], bv1[0:1, :], start=False, stop=True
                )
                t, hv = St // 2, St % 2
                nc.vector.tensor_copy(
                    Vt[t][:, :, hv, 0:64],
                    ps.rearrange("p (c d) -> p c d", c=8),
                )

            # Upfront projections double as the HAM warm-up: 24 back-to-back
            # matmuls from DMA-land keep the PE busy through the cold window.
            proj_todo = deque()
            proj_q(0)
            proj_k(0)
            proj_q(1)
            proj_k(1)
            proj_todo.append(lambda: proj_v(0))
            proj_todo.append(lambda: proj_v(1))
            proj_todo.append(lambda: proj_k(2))
            proj_todo.append(lambda: proj_k(3))
            proj_todo.append(lambda: proj_q(2))
            proj_todo.append(lambda: proj_q(3))
            proj_todo.append(lambda: proj_v(2))
            proj_todo.append(lambda: proj_v(3))

            # ---- attention: 4 phases x 16 slots; slot = (n, jp) n-major ----
            phases = [(0, 0), (1, 0), (0, 1), (1, 1)]
            LAG = 2
            pend = deque()
            extract_pending = deque()
            merge_q = deque()
            O_tiles = {}

            def do_merge(H, ii, on_act=False):
                mp = pp.tile([128, 512], f32, tag="pp", name="mp")
                nc.tensor.matmul(mp, OT[H][:, 128 * ii : 128 * ii + 128], wm_sb)
                ms = msp.tile([128, 512], bf16, tag="ms", name="ms")
                if on_act:
                    nc.scalar.activation(ms, mp, Copy, scale=1.0)
                else:
                    nc.vector.tensor_copy(ms, mp)
                j0 = 128 * (ii % 2)
                nc.sync.dma_start(
                    out=d_out_r[j0 : j0 + 128, 4 * H + ii // 2, :], in_=ms
                )

            def do_extract(pi, n):
                # copy BOTH psum accumulators out first so the O banks free
                # quickly for the next q-half's PV.
                t, H = phases[pi]
                Oa, Ob = O_tiles[(pi, n)]
                ca = osp.tile([65, 512], f32, tag="ca", name="ca")
                nc.vector.tensor_copy(ca, Oa[0:65, :])
                cb = osp.tile([65, 512], f32, tag="cb", name="cb")
                nc.vector.tensor_copy(cb, Ob[0:65, :])
                dsum = drp.tile([1, 512], f32, tag="ds", name="dsum")
                nc.vector.tensor_tensor(dsum, cb[64:65, :], ca[64:65, :], add)
                rr = drp.tile([1, 512], f32, tag="rr", name="rr")
                nc.vector.reciprocal_approx_fast(rr, dsum)
                rb = rcp.tile([64, 512], f32, tag="rb", name="rb")
                nc.gpsimd.partition_broadcast(rb, rr)
                osum = osp.tile([64, 512], f32, tag="os", name="osum")
                nc.vector.tensor_tensor(osum, cb[0:64, :], ca[0:64, :], add)
                ots = OT[H][64 * t : 64 * t + 64, 512 * n : 512 * n + 512]
                nc.vector.tensor_tensor(ots, osum, rb, mult)
                if t == 1:
                    last = pi == 3 and n == 1
                    for ii in range(4 * n, 4 * n + 4):
                        merge_q.append(
                            lambda ii=ii, H=H, a=last: do_merge(H, ii, on_act=a)
                        )

            def issue_pv(pi, n, jp, pt):
                t, H = phases[pi]
                if (pi, n) not in O_tiles:
                    O_tiles[(pi, n)] = (
                        op.tile([65, 512], f32, tag="op", name="Oa"),
                        op.tile([65, 512], f32, tag="op", name="Ob"),
                    )
                Oa, Ob = O_tiles[(pi, n)]
                for ci in (0, 1):
                    jj = jp + 8 * ci
                    c, hv = jj // 2, jj % 2
                    st = jp == 0 and ci == 0
                    sp_ = jp == 7 and ci == 1
                    nc.tensor.matmul(
                        Oa[0:65, :],
                        Vt[t][0:64, c, hv, :],
                        pt[0:64, 512 * ci : 512 * ci + 512],
                        start=st,
                        stop=sp_,
                    )
                    nc.tensor.matmul(
                        Ob[0:65, :],
                        Vt[t][64:128, c, hv, :],
                        pt[64:128, 512 * ci : 512 * ci + 512],
                        start=st,
                        stop=sp_,
                    )
                if jp == 7:
                    extract_pending.append((pi, n))

            for pi, (t, H) in enumerate(phases):
                for s in range(16):
                    n, jp = s // 8, s % 8
                    sT = sp.tile([128, 1024], f32, tag="sp", name="sT")
                    nc.tensor.matmul(
                        sT[:, 0:512],
                        KT2[0:64, t, 128 * jp : 128 * jp + 128],
                        QQ[0:64, t, 1024 * H + 512 * n : 1024 * H + 512 * n + 512],
                    )
                    nc.tensor.matmul(
                        sT[:, 512:1024],
                        KT2[64:128, t, 128 * jp : 128 * jp + 128],
                        QQ[64:128, t, 1024 * H + 512 * n : 1024 * H + 512 * n + 512],
                    )
                    pt = ptp.tile([128, 1024], bf16, tag="pt")
                    nc.scalar.activation(pt, sT, Exp, scale=0.125)
                    if proj_todo and (pi + s < 4 or s % 2 == 0):
                        proj_todo.popleft()()
                    while extract_pending:
                        do_extract(*extract_pending.popleft())
                    pend.append((pi, n, jp, pt))
                    if pend and pend[0][0] < pi:
                        issue_pv(*pend.popleft())
                    if len(pend) > LAG:
                        issue_pv(*pend.popleft())
                    if pi == 3 and s >= 12 and pend:
                        issue_pv(*pend.popleft())
                    if merge_q and (pi == 3 or s < 14):
                        merge_q.popleft()()
                    if pi == 3 and merge_q:
                        merge_q.popleft()()
            while pend:
                issue_pv(*pend.popleft())
                while extract_pending:
                    do_extract(*extract_pending.popleft())
            while extract_pending:
                do_extract(*extract_pending.popleft())
            while merge_q:
                merge_q.popleft()()

    nc.finalize()
    _CACHE["nc"] = nc
    return nc


def _prep_in_maps(q, k, v, Wq, Wk, Wv, Wm, bq, bk, bv):
    import ml_dtypes

    f = np.float32
    b16 = ml_dtypes.bfloat16

    def cont(a):
        # [512, C] -> [128, 4*C]: row t*128+p -> partition p, block t
        C = a.shape[1]
        return np.ascontiguousarray(
            a.reshape(4, 128, C).transpose(1, 0, 2).reshape(128, 4 * C)
        )

    WqT = Wq.T.astype(b16)  # plain [512, 512]
    # wkT: columns paired (c, c+4) -> col 128*cp + 64*h + d = orig (cp+4h)*64+d
    WkT = Wk.T.astype(b16)
    WkT = np.ascontiguousarray(
        WkT.reshape(512, 2, 4, 64).transpose(0, 2, 1, 3).reshape(512, 512)
    )
    WvT = np.ascontiguousarray(Wv.T.astype(b16))
    WmT = Wm.T.astype(f)  # [e_in, e_out]

    t_ = np.ascontiguousarray(bq.astype(f).reshape(8, 64).T)  # [64, 8]
    bq8 = np.ascontiguousarray(np.vstack([t_, t_]))  # [128, 8]
    kk = bk.astype(f).reshape(8, 64)
    bk4 = np.ascontiguousarray(np.concatenate([kk[0:4].T, kk[4:8].T], axis=0))
    bv1 = np.ascontiguousarray(bv.astype(b16).reshape(1, 512))

    in_maps = []
    for c in range(NCORES):
        b_, j4 = c // 4, c % 4
        r0 = 512 * j4
        m = {
            "xqT": cont(q[b_, r0 : r0 + 512, :].T.astype(b16)),
            "xkT": cont(k[b_, r0 : r0 + 512, :].T.astype(b16)),
            "xvT": cont(v[b_, r0 : r0 + 512, :].T.astype(b16)),
            "wqT": cont(WqT),
            "wkT": cont(WkT),
            "wvT": cont(WvT),
            "wmP": np.ascontiguousarray(
                WmT[128 * j4 : 128 * j4 + 128, :].astype(b16)
            ),
            "bq8": bq8,
            "bk4": bk4,
            "bv1": bv1,
            "ones1": np.ones((1, 128), b16),
        }
        in_maps.append(m)
    return in_maps


def _reference_fallback(q, k, v, mask, Wq, Wk, Wv, Wm, bq, bk, bv, bm):
    # Only used if mask is nonzero (spec fills it with zeros).
    f = np.float32
    qh = (q.astype(f) @ Wq.T.astype(f) + bq).reshape(B, 8, S, DHEAD)
    kh = (k.astype(f) @ Wk.T.astype(f) + bk).reshape(B, 8, S, DHEAD)
    vh = (v.astype(f) @ Wv.T.astype(f) + bv).reshape(B, 8, S, DHEAD)
    s = np.einsum("bhqd,bhkd->bhqk", qh, kh) / np.sqrt(np.float32(DHEAD))
    s = np.where(mask, np.float32(-1e9), s)
    s = s - s.max(-1, keepdims=True)
    e = np.exp(s)
    p = e / e.sum(-1, keepdims=True)
    attn = np.einsum("bhqk,bhkd->bhqd", p, vh)
    attn = attn.transpose(0, 2, 1, 3).reshape(B, S, HIDDEN)
    return attn @ Wm.T.astype(f) + bm


def kernel(q, k, v, mask, Wq, Wk, Wv, Wm, bq, bk, bv, bm):
    global LAST_EXEC_NS, LAST_RESULTS
    q, k, v = (np.asarray(a, np.float32) for a in (q, k, v))
    mask = np.asarray(mask)
    Wq, Wk, Wv, Wm = (np.asarray(a, np.float32) for a in (Wq, Wk, Wv, Wm))
    bq, bk, bv, bm = (np.asarray(a, np.float32) for a in (bq, bk, bv, bm))

    if mask.any():
        return _reference_fallback(q, k, v, mask, Wq, Wk, Wv, Wm, bq, bk, bv, bm)

    from concourse.bass_utils import run_bass_kernel_spmd

    nc = _build_nc()
    in_maps = _prep_in_maps(q, k, v, Wq, Wk, Wv, Wm, bq, bk, bv)
    res = run_bass_kernel_spmd(nc, in_maps, list(range(NCORES)))
    LAST_RESULTS = res
    LAST_EXEC_NS = getattr(res, "exec_time_ns", None)

    out = np.zeros((B, S, HIDDEN), np.float32)
    for c in range(NCORES):
        out[c // 4] += res.results[c]["outp"].astype(np.float32)
    out += bm
    return out
